# revision 1
# baseline (speedup 1.0000x reference)
"""Trainium2 Bass kernel for nn_Jointer: per-sample masked cosine-similarity.

out[b] = relu(l2norm(source[b]) @ l2norm(target[b]).T) * (mask_src[b] outer mask_tar[b])

Sharding: data-parallel over batch B=8 -> one sample per NeuronCore.
Per core: normalize+mask fold, PE-transpose both operands to [D, tokens],
fp32r matmul in 128x512 tiles, fused scale+relu out of PSUM, 1MB row DMAs.
"""

import numpy as np

import concourse.bass as bass
from concourse import bacc
import concourse.mybir as mybir
import concourse.tile as tile
from concourse.bass_utils import run_bass_kernel_spmd
from concourse.masks import make_identity

F32 = mybir.dt.float32
F32R = mybir.dt.float32r
AF = mybir.ActivationFunctionType
ALU = mybir.AluOpType

S = 2048  # source tokens per sample
T = 2048  # target tokens per sample
D = 128  # feature dim (= contraction dim = partitions)
P = 128  # partitions
SB = S // P  # 16 source token blocks
TB = T // P  # 16 target token blocks
NT = 512  # matmul moving free dim (one PSUM bank of fp32)
NCHUNKS = T // NT  # 4


def build_nc() -> bass.Bass:
    nc = bacc.Bacc(trn_type="TRN2")

    src = nc.dram_tensor("src", [S, D], F32, kind="ExternalInput")
    tgt = nc.dram_tensor("tgt", [T, D], F32, kind="ExternalInput")
    # maskf[p, k]: k in [0,16) source-block masks, k in [16,32) target-block
    # masks; value for token 128*k + p.
    maskf = nc.dram_tensor("maskf", [P, SB + TB], F32, kind="ExternalInput")
    out = nc.dram_tensor("out", [S, T], F32, kind="ExternalOutput")

    src_r = src.rearrange("(k p) d -> p k d", p=P)
    tgt_r = tgt.rearrange("(k p) d -> p k d", p=P)
    out_r = out.rearrange("(m p) n -> m p n", p=P)
    mask_r = maskf.rearrange("p k -> p k")

    G = 4  # blocks per pipeline group
    NG = TB // G  # 4 groups

    with tile.TileContext(nc) as tc:
        with (
            tc.tile_pool(name="singles", bufs=1) as singles,
            tc.tile_pool(name="inbuf", bufs=1) as inbuf,
            tc.tile_pool(name="sq", bufs=2) as sqpool,
            tc.tile_pool(name="norm", bufs=1) as normp,
            tc.tile_pool(name="tscl", bufs=3) as tsclp,
            tc.tile_pool(name="pst", bufs=2, space="PSUM") as psum_t,
            tc.tile_pool(name="psmm", bufs=4, space="PSUM") as psum_mm,
            tc.tile_pool(name="outp", bufs=4) as outp,
        ):
            ident = singles.tile([P, P], F32)
            make_identity(nc, ident)

            mask_sb = singles.tile([P, SB + TB], F32)
            nc.sync.dma_start(out=mask_sb, in_=mask_r)

            s_nat = inbuf.tile([P, SB, D], F32)
            sT = inbuf.tile([P, S], F32R)  # [D, s tokens] (raw)
            s_scl = normp.tile([P, SB], F32)
            t_nat = inbuf.tile([P, TB, D], F32)
            tT = inbuf.tile([P, T], F32R)  # [D, t tokens] normalized+masked
            t_scl = normp.tile([P, TB], F32)

            def s_load(g):
                blk = slice(g * G, (g + 1) * G)
                nc.sync.dma_start(out=s_nat[:, blk, :], in_=src_r[:, blk, :])
                ps = psum_t.tile([P, G * P], F32, tag="pst", name=f"ps_s{g}")
                for j in range(G):
                    k = g * G + j
                    nc.tensor.transpose(
                        ps[:, j * P : (j + 1) * P], s_nat[:, k, :], ident
                    )
                nc.vector.tensor_copy(
                    out=sT[:, g * G * P : (g + 1) * G * P], in_=ps
                )

            def s_norm(g):
                blk = slice(g * G, (g + 1) * G)
                s_sq = sqpool.tile([P, G, D], F32, tag="sq", name=f"ssq{g}")
                nc.scalar.activation(out=s_sq, in_=s_nat[:, blk, :], func=AF.Square)
                s_ss = normp.tile([P, G], F32, tag="sss", name=f"sss{g}")
                nc.vector.reduce_sum(out=s_ss, in_=s_sq, axis=mybir.AxisListType.X)
                s_rcp = normp.tile([P, G], F32, tag="srcp", name=f"srcp{g}")
                nc.vector.reciprocal(out=s_rcp, in_=s_ss)
                s_rsq = normp.tile([P, G], F32, tag="srsq", name=f"srsq{g}")
                nc.scalar.activation(out=s_rsq, in_=s_rcp, func=AF.Sqrt)
                nc.vector.tensor_mul(
                    out=s_scl[:, blk],
                    in0=s_rsq,
                    in1=mask_sb[:, g * G : (g + 1) * G],
                )

            t_rsqs = {}

            def t_norm(g):
                blk = slice(g * G, (g + 1) * G)
                nc.sync.dma_start(out=t_nat[:, blk, :], in_=tgt_r[:, blk, :])
                t_sq = sqpool.tile([P, G, D], F32, tag="sq", name=f"tsq{g}")
                nc.scalar.activation(out=t_sq, in_=t_nat[:, blk, :], func=AF.Square)
                t_ss = normp.tile([P, G], F32, tag="tss", name=f"tss{g}")
                nc.vector.reduce_sum(out=t_ss, in_=t_sq, axis=mybir.AxisListType.X)
                t_rcp = normp.tile([P, G], F32, tag="trcp", name=f"trcp{g}")
                nc.vector.reciprocal(out=t_rcp, in_=t_ss)
                t_rsq = normp.tile([P, G], F32, tag="trsq", name=f"trsq{g}")
                nc.scalar.activation(out=t_rsq, in_=t_rcp, func=AF.Sqrt)
                t_rsqs[g] = t_rsq

            def t_xpose(g):
                # scale*mask + transpose 4 blocks; two half-bank copies run on
                # ACT and DVE in parallel to cut the chain latency.
                t_rsq = t_rsqs[g]
                ps = psum_t.tile([P, G * P], F32, tag="pst", name=f"ps_t{g}")
                for j in range(G):
                    k = g * G + j
                    t_sc = tsclp.tile([P, P], F32, tag="tscl")
                    nc.vector.tensor_scalar(
                        out=t_sc,
                        in0=t_nat[:, k, :],
                        scalar1=t_rsq[:, j : j + 1],
                        scalar2=mask_sb[:, SB + k : SB + k + 1],
                        op0=ALU.mult,
                        op1=ALU.mult,
                    )
                    nc.tensor.transpose(ps[:, j * P : (j + 1) * P], t_sc, ident)
                half = G * P // 2
                base = g * G * P
                nc.scalar.copy(out=tT[:, base : base + half], in_=ps[:, 0:half])
                nc.vector.tensor_copy(
                    out=tT[:, base + half : base + 2 * half],
                    in_=ps[:, half : 2 * half],
                )

            # --- main matmul + fused (scale * relu) + output DMA.
            # First rows stream per-512-chunk DMAs so the DMA queue saturates
            # as soon as the first tT chunk lands; later rows use 1MB row DMAs.
            EARLY_ROWS = 2
            ob_tiles = {}

            def mm_chunk(m, n):
                if m not in ob_tiles:
                    ob_tiles[m] = outp.tile([P, T], F32, tag="ob", name=f"ob{m}")
                ob = ob_tiles[m]
                ps = psum_mm.tile([P, NT], F32, tag="psmm", name=f"mm{m}_{n}")
                nc.tensor.matmul(
                    ps,
                    sT[:, m * P : (m + 1) * P],
                    tT[:, n * NT : (n + 1) * NT],
                    start=True,
                    stop=True,
                )
                dst = ob[:, n * NT : (n + 1) * NT]
                if (m * NCHUNKS + n) % 2 == 0:
                    nc.scalar.activation(
                        out=dst, in_=ps, func=AF.Relu, scale=s_scl[:, m : m + 1]
                    )
                else:
                    nc.vector.tensor_scalar(
                        out=dst,
                        in0=ps,
                        scalar1=s_scl[:, m : m + 1],
                        scalar2=0.0,
                        op0=ALU.mult,
                        op1=ALU.max,
                    )
                if m < EARLY_ROWS:
                    nc.sync.dma_start(
                        out=out_r[m][:, n * NT : (n + 1) * NT], in_=dst
                    )
                elif n == NCHUNKS - 1:
                    nc.sync.dma_start(out=out_r[m], in_=ob)

            def mm_row(m):
                for n in range(NCHUNKS):
                    mm_chunk(m, n)

            # Emission order == per-engine FIFO order, so it must match data
            # readiness: t0's norm chain leads the ACT/DVE FIFOs (it is the
            # critical path to the first output chunk), s0's transposes lead
            # the PE FIFO (their data lands first), and row-0 chunks
            # interleave with the t groups that feed them.  Remaining s
            # groups fill engine gaps between row batches.
            t_norm(0)
            s_load(0)
            t_xpose(0)
            s_norm(0)
            mm_chunk(0, 0)
            t_norm(1)
            t_xpose(1)
            mm_chunk(0, 1)
            t_norm(2)
            t_xpose(2)
            mm_chunk(0, 2)
            t_norm(3)
            t_xpose(3)
            mm_chunk(0, 3)
            mm_row(1)
            s_load(1)
            mm_row(2)
            s_norm(1)
            mm_row(3)
            s_load(2)
            mm_row(4)
            s_norm(2)
            mm_row(5)
            mm_row(6)
            s_load(3)
            mm_row(7)
            s_norm(3)
            for m in range(8, 16):
                mm_row(m)

    nc.compile()
    return nc


_NC_CACHE = None


def _get_nc():
    global _NC_CACHE
    if _NC_CACHE is None:
        _NC_CACHE = build_nc()
    return _NC_CACHE


def kernel(source, target, mask_src, mask_tar, **run_kwargs):
    source = np.asarray(source, dtype=np.float32)
    target = np.asarray(target, dtype=np.float32)
    mask_src = np.asarray(mask_src)
    mask_tar = np.asarray(mask_tar)
    B = source.shape[0]

    in_maps = []
    for b in range(B):
        msf = mask_src[b].astype(np.float32).reshape(SB, P).T
        mtf = mask_tar[b].astype(np.float32).reshape(TB, P).T
        mk = np.ascontiguousarray(np.concatenate([msf, mtf], axis=1))
        in_maps.append(
            {
                "src": np.ascontiguousarray(source[b]),
                "tgt": np.ascontiguousarray(target[b]),
                "maskf": mk,
            }
        )

    nc = _get_nc()
    res = run_bass_kernel_spmd(nc, in_maps, core_ids=list(range(B)), **run_kwargs)
    out = np.stack([r["out"] for r in res.results], axis=0)
    if run_kwargs.get("trace"):
        kernel.last_results = res
    return out



# revision 2
# speedup vs baseline: 1.3868x; 1.3868x over previous
"""Trainium2 Bass kernel for nn_Jointer: per-sample masked cosine-similarity.

out[b] = relu(l2norm(source[b]) @ l2norm(target[b]).T) * (mask_src[b] outer mask_tar[b])

Sharding: data-parallel over batch B=8 -> one sample per NeuronCore.
Per core: fp32 norms, PE-transpose both operands to [D, tokens] bf16,
bf16 matmul in 128x512 tiles, fused scale+relu out of PSUM to fp16,
fp16 row DMAs (half the HBM write traffic of fp32).
"""

import numpy as np

import concourse.bass as bass
from concourse import bacc
import concourse.mybir as mybir
import concourse.tile as tile
from concourse.bass_utils import run_bass_kernel_spmd
from concourse.masks import make_identity

F32 = mybir.dt.float32
BF16 = mybir.dt.bfloat16
F16 = mybir.dt.float16
AF = mybir.ActivationFunctionType
ALU = mybir.AluOpType

S = 2048  # source tokens per sample
T = 2048  # target tokens per sample
D = 128  # feature dim (= contraction dim = partitions)
P = 128  # partitions
SB = S // P  # 16 source token blocks
TB = T // P  # 16 target token blocks
NT = 512  # matmul moving free dim (one PSUM bank of fp32)
NCHUNKS = T // NT  # 4
G = 4  # blocks per pipeline group
NG = TB // G  # 4 groups


def build_nc() -> bass.Bass:
    nc = bacc.Bacc(trn_type="TRN2")

    src = nc.dram_tensor("src", [S, D], F32, kind="ExternalInput")
    tgt = nc.dram_tensor("tgt", [T, D], F32, kind="ExternalInput")
    # maskf[p, k]: k in [0,16) source-block masks (token p*16+k),
    # k in [16,32) target-block masks (token (k-16)*128+p).
    maskf = nc.dram_tensor("maskf", [P, SB + TB], F32, kind="ExternalInput")
    out = nc.dram_tensor("out", [S, T], F16, kind="ExternalOutput")

    # source tokens in (p k) order: token p*16+k -> partition p, block k.
    # Per-partition DRAM lines are 16*128*4B = 8KB contiguous.
    src_r = src.rearrange("(p k) d -> p k d", p=P)
    # target tokens in (k p) order: token k*128+p -> partition p, block k,
    # so transposed tT columns are in natural token order.
    tgt_r = tgt.rearrange("(k p) d -> p k d", p=P)
    # out row-block m holds rows {p*16+m}: partition p -> DRAM row p*16+m.
    out_r = out.rearrange("(p m) n -> m p n", m=SB)

    with tile.TileContext(nc) as tc:
        with (
            tc.tile_pool(name="singles", bufs=1) as singles,
            tc.tile_pool(name="inbuf", bufs=1) as inbuf,
            tc.tile_pool(name="sq", bufs=2) as sqpool,
            tc.tile_pool(name="norm", bufs=1) as normp,
            tc.tile_pool(name="tscl", bufs=3) as tsclp,
            tc.tile_pool(name="pst", bufs=2, space="PSUM") as psum_t,
            tc.tile_pool(name="psmm", bufs=3, space="PSUM") as psum_mm,
            tc.tile_pool(name="outp", bufs=4) as outp,
        ):
            ident = singles.tile([P, P], F32)
            make_identity(nc, ident)

            mask_sb = singles.tile([P, SB + TB], F32)
            nc.sync.dma_start(out=mask_sb, in_=maskf.rearrange("p k -> p k"))

            s_nat = inbuf.tile([P, SB, D], F32)
            sT = inbuf.tile([P, S], BF16)  # [D, s tokens] (raw, bf16)
            s_scl = normp.tile([P, SB], F32)  # rsqrt(|s|^2)*mask per token
            t_nat = inbuf.tile([P, TB, D], F32)
            tT = inbuf.tile([P, T], BF16)  # [D, t tokens] normalized+masked

            def s_load(g):
                blk = slice(g * G, (g + 1) * G)
                nc.sync.dma_start(out=s_nat[:, blk, :], in_=src_r[:, blk, :])

            def t_load(g):
                blk = slice(g * G, (g + 1) * G)
                nc.sync.dma_start(out=t_nat[:, blk, :], in_=tgt_r[:, blk, :])

            def s_xpose(g):
                # raw fp32 transpose; downconvert to bf16 in the PSUM copy.
                ps = psum_t.tile([P, G * P], F32, tag="pst", name=f"ps_s{g}")
                for j in range(G):
                    k = g * G + j
                    nc.tensor.transpose(
                        ps[:, j * P : (j + 1) * P], s_nat[:, k, :], ident
                    )
                nc.vector.tensor_copy(
                    out=sT[:, g * G * P : (g + 1) * G * P], in_=ps
                )

            def s_norm(g):
                blk = slice(g * G, (g + 1) * G)
                s_sq = sqpool.tile([P, G, D], F32, tag="sq", name=f"ssq{g}")
                nc.scalar.activation(out=s_sq, in_=s_nat[:, blk, :], func=AF.Square)
                s_ss = normp.tile([P, G], F32, tag="sss", name=f"sss{g}")
                nc.vector.reduce_sum(out=s_ss, in_=s_sq, axis=mybir.AxisListType.X)
                s_rcp = normp.tile([P, G], F32, tag="srcp", name=f"srcp{g}")
                nc.vector.reciprocal(out=s_rcp, in_=s_ss)
                s_rsq = normp.tile([P, G], F32, tag="srsq", name=f"srsq{g}")
                nc.scalar.activation(out=s_rsq, in_=s_rcp, func=AF.Sqrt)
                nc.vector.tensor_mul(
                    out=s_scl[:, blk],
                    in0=s_rsq,
                    in1=mask_sb[:, g * G : (g + 1) * G],
                )

            t_scls = {}

            def t_norm(g):
                blk = slice(g * G, (g + 1) * G)
                t_sq = sqpool.tile([P, G, D], F32, tag="sq", name=f"tsq{g}")
                nc.scalar.activation(out=t_sq, in_=t_nat[:, blk, :], func=AF.Square)
                t_ss = normp.tile([P, G], F32, tag="tss", name=f"tss{g}")
                nc.vector.reduce_sum(out=t_ss, in_=t_sq, axis=mybir.AxisListType.X)
                t_rcp = normp.tile([P, G], F32, tag="trcp", name=f"trcp{g}")
                nc.vector.reciprocal(out=t_rcp, in_=t_ss)
                t_rsq = normp.tile([P, G], F32, tag="trsq", name=f"trsq{g}")
                nc.scalar.activation(out=t_rsq, in_=t_rcp, func=AF.Sqrt)
                t_scl = normp.tile([P, G], F32, tag="tscl2", name=f"tscl2_{g}")
                nc.vector.tensor_mul(
                    out=t_scl,
                    in0=t_rsq,
                    in1=mask_sb[:, SB + g * G : SB + (g + 1) * G],
                )
                t_scls[g] = t_scl

            def t_xpose(g):
                # scale*mask (single fused scalar) then fp32 transpose;
                # downconvert to bf16 in the PSUM copy.
                t_scl = t_scls[g]
                ps = psum_t.tile([P, G * P], F32, tag="pst", name=f"ps_t{g}")
                for j in range(G):
                    k = g * G + j
                    t_sc = tsclp.tile([P, P], F32, tag="tscl")
                    nc.vector.tensor_scalar(
                        out=t_sc,
                        in0=t_nat[:, k, :],
                        scalar1=t_scl[:, j : j + 1],
                        scalar2=None,
                        op0=ALU.mult,
                    )
                    nc.tensor.transpose(ps[:, j * P : (j + 1) * P], t_sc, ident)
                half = G * P // 2
                base = g * G * P
                nc.scalar.copy(out=tT[:, base : base + half], in_=ps[:, 0:half])
                nc.vector.tensor_copy(
                    out=tT[:, base + half : base + 2 * half],
                    in_=ps[:, half : 2 * half],
                )

            # --- main matmul + fused (scale * relu) -> fp16 + output DMA.
            # Two MMs fill a 2-bank PSUM tile; one 1024-wide copy drains it.
            # Engine pattern gives ACT ~60% of the copies (ACT is faster
            # per element out of PSUM and has less other work).
            ACT_COPY = (0, 1, 3)  # of every 4 half-row copies, these on ACT

            def mm_row(m, ob):
                for h in range(2):  # two half-rows of 1024
                    ps = psum_mm.tile([P, 2 * NT], F32, tag="psmm", name=f"mm{m}_{h}")
                    for q in range(2):
                        n = 2 * h + q
                        nc.tensor.matmul(
                            ps[:, q * NT : (q + 1) * NT],
                            sT[:, m * P : (m + 1) * P],
                            tT[:, n * NT : (n + 1) * NT],
                            start=True,
                            stop=True,
                        )
                    dst = ob[:, h * 2 * NT : (h + 1) * 2 * NT]
                    if (2 * m + h) % 4 in ACT_COPY:
                        nc.scalar.activation(
                            out=dst, in_=ps, func=AF.Relu, scale=s_scl[:, m : m + 1]
                        )
                    else:
                        nc.vector.tensor_scalar(
                            out=dst,
                            in0=ps,
                            scalar1=s_scl[:, m : m + 1],
                            scalar2=0.0,
                            op0=ALU.mult,
                            op1=ALU.max,
                        )

            def out_row(m):
                ob = outp.tile([P, T], F16, tag="ob", name=f"ob{m}")
                mm_row(m, ob)
                nc.sync.dma_start(out=out_r[m], in_=ob)

            # Emission order == per-engine FIFO order. The t pipeline is the
            # critical path to the first full output row (row DMAs need all
            # 4 tT chunks), so t groups lead; s group 0 (transpose + norms)
            # interleaves so row 0 can start immediately after tT completes;
            # remaining s groups fill gaps between row batches.
            t_load(0)
            s_load(0)
            t_load(1)
            t_load(2)
            t_load(3)
            t_norm(0)
            s_xpose(0)
            t_xpose(0)
            s_norm(0)
            t_norm(1)
            t_xpose(1)
            t_norm(2)
            t_xpose(2)
            t_norm(3)
            t_xpose(3)
            out_row(0)
            out_row(1)
            s_load(1)
            out_row(2)
            s_xpose(1)
            out_row(3)
            s_norm(1)
            out_row(4)
            s_load(2)
            out_row(5)
            s_xpose(2)
            out_row(6)
            s_norm(2)
            out_row(7)
            s_load(3)
            out_row(8)
            s_xpose(3)
            out_row(9)
            s_norm(3)
            for m in range(10, SB):
                out_row(m)

    nc.compile()
    return nc


_NC_CACHE = None


def _get_nc():
    global _NC_CACHE
    if _NC_CACHE is None:
        _NC_CACHE = build_nc()
    return _NC_CACHE


def kernel(source, target, mask_src, mask_tar, **run_kwargs):
    source = np.asarray(source, dtype=np.float32)
    target = np.asarray(target, dtype=np.float32)
    mask_src = np.asarray(mask_src)
    mask_tar = np.asarray(mask_tar)
    B = source.shape[0]

    in_maps = []
    for b in range(B):
        # source tokens in (p k) order; target tokens in (k p) order.
        msf = mask_src[b].astype(np.float32).reshape(P, SB)
        mtf = mask_tar[b].astype(np.float32).reshape(TB, P).T
        mk = np.ascontiguousarray(np.concatenate([msf, mtf], axis=1))
        in_maps.append(
            {
                "src": np.ascontiguousarray(source[b]),
                "tgt": np.ascontiguousarray(target[b]),
                "maskf": mk,
            }
        )

    nc = _get_nc()
    res = run_bass_kernel_spmd(nc, in_maps, core_ids=list(range(B)), **run_kwargs)
    out = np.stack(
        [np.asarray(r["out"], dtype=np.float32) for r in res.results], axis=0
    )
    if run_kwargs.get("trace"):
        kernel.last_results = res
    return out


# revision 4
# speedup vs baseline: 1.4504x; 1.0459x over previous
"""Trainium2 Bass kernel for nn_Jointer: per-sample masked cosine-similarity.

out[b] = relu(l2norm(source[b]) @ l2norm(target[b]).T) * (mask_src[b] outer mask_tar[b])

Sharding: data-parallel over batch B=8 -> one sample per NeuronCore.
Per core: fp32 norms (batched per half-side), PE-transpose both operands
to [D, tokens] bf16, bf16 matmul, fused scale+relu out of PSUM to fp16,
half-row (256KB) output DMAs so the write stream starts early.
"""

import numpy as np

import concourse.bass as bass
from concourse import bacc
import concourse.mybir as mybir
import concourse.tile as tile
from concourse.bass_utils import run_bass_kernel_spmd
from concourse.masks import make_identity

F32 = mybir.dt.float32
BF16 = mybir.dt.bfloat16
F16 = mybir.dt.float16
AF = mybir.ActivationFunctionType
ALU = mybir.AluOpType

S = 2048  # source tokens per sample
T = 2048  # target tokens per sample
D = 128  # feature dim (= contraction dim = partitions)
P = 128  # partitions
SB = S // P  # 16 source token blocks
TB = T // P  # 16 target token blocks
NT = 512  # matmul moving free dim (one PSUM bank of fp32)
G = 4  # blocks per transpose group
HB = TB // 2  # 8 blocks per half-side


def build_nc() -> bass.Bass:
    nc = bacc.Bacc(trn_type="TRN2")

    src = nc.dram_tensor("src", [S, D], F32, kind="ExternalInput")
    tgt = nc.dram_tensor("tgt", [T, D], F32, kind="ExternalInput")
    # maskf[p, k]: k in [0,16) source-block masks (token p*16+k),
    # k in [16,32) target-block masks (token (k-16)*128+p).
    maskf = nc.dram_tensor("maskf", [P, SB + TB], F32, kind="ExternalInput")
    out = nc.dram_tensor("out", [S, T], F16, kind="ExternalOutput")

    # source tokens in (p k) order: token p*16+k -> partition p, block k.
    # Per-partition DRAM lines are 16*128*4B = 8KB contiguous.
    src_r = src.rearrange("(p k) d -> p k d", p=P)
    # target tokens in (k p) order: token k*128+p -> partition p, block k,
    # so transposed tT columns are in natural token order.
    tgt_r = tgt.rearrange("(k p) d -> p k d", p=P)
    # out row-block m holds rows {p*16+m}: partition p -> DRAM row p*16+m.
    out_r = out.rearrange("(p m) n -> m p n", m=SB)

    with tile.TileContext(nc) as tc:
        with (
            tc.tile_pool(name="singles", bufs=1) as singles,
            tc.tile_pool(name="inbuf", bufs=1) as inbuf,
            tc.tile_pool(name="norm", bufs=1) as normp,
            tc.tile_pool(name="pst", bufs=2, space="PSUM") as psum_t,
            tc.tile_pool(name="psmm", bufs=3, space="PSUM") as psum_mm,
            tc.tile_pool(name="outp", bufs=6) as outp,
        ):
            ident = singles.tile([P, P], F32)
            make_identity(nc, ident)

            mask_sb = singles.tile([P, SB + TB], F32)
            nc.sync.dma_start(out=mask_sb, in_=maskf.rearrange("p k -> p k"))

            s_nat = inbuf.tile([P, SB, D], F32)
            sT = inbuf.tile([P, S], BF16)  # [D, s tokens] (raw, bf16)
            s_scl = normp.tile([P, SB], F32)  # rsqrt(|s|^2)*mask per token
            s_sq = inbuf.tile([P, SB, D], F32)
            t_nat = inbuf.tile([P, TB, D], F32)
            t_sc = inbuf.tile([P, TB, D], F32)  # normalized+masked target
            t_sq = inbuf.tile([P, TB, D], F32)
            tT = inbuf.tile([P, T], BF16)  # [D, t tokens] normalized+masked

            def s_load(g):
                blk = slice(g * G, (g + 1) * G)
                nc.sync.dma_start(out=s_nat[:, blk, :], in_=src_r[:, blk, :])

            def t_load(g):
                blk = slice(g * G, (g + 1) * G)
                nc.sync.dma_start(out=t_nat[:, blk, :], in_=tgt_r[:, blk, :])

            def s_xpose(g):
                # raw fp32 transpose; downconvert to bf16 in the PSUM copy.
                ps = psum_t.tile([P, G * P], F32, tag="pst", name=f"ps_s{g}")
                for j in range(G):
                    k = g * G + j
                    nc.tensor.transpose(
                        ps[:, j * P : (j + 1) * P], s_nat[:, k, :], ident
                    )
                nc.vector.tensor_copy(
                    out=sT[:, g * G * P : (g + 1) * G * P], in_=ps
                )

            def s_norm(h):
                # batched norm chain for half-side h (8 blocks).
                blk = slice(h * HB, (h + 1) * HB)
                nc.scalar.activation(
                    out=s_sq[:, blk, :], in_=s_nat[:, blk, :], func=AF.Square
                )
                s_ss = normp.tile([P, HB], F32, tag="sss", name=f"sss{h}")
                nc.vector.reduce_sum(
                    out=s_ss, in_=s_sq[:, blk, :], axis=mybir.AxisListType.X
                )
                s_rcp = normp.tile([P, HB], F32, tag="srcp", name=f"srcp{h}")
                nc.vector.reciprocal(out=s_rcp, in_=s_ss)
                s_rsq = normp.tile([P, HB], F32, tag="srsq", name=f"srsq{h}")
                nc.scalar.activation(out=s_rsq, in_=s_rcp, func=AF.Sqrt)
                nc.vector.tensor_mul(
                    out=s_scl[:, blk],
                    in0=s_rsq,
                    in1=mask_sb[:, h * HB : (h + 1) * HB],
                )

            def t_norm(h):
                blk = slice(h * HB, (h + 1) * HB)
                nc.scalar.activation(
                    out=t_sq[:, blk, :], in_=t_nat[:, blk, :], func=AF.Square
                )
                t_ss = normp.tile([P, HB], F32, tag="tss", name=f"tss{h}")
                nc.vector.reduce_sum(
                    out=t_ss, in_=t_sq[:, blk, :], axis=mybir.AxisListType.X
                )
                t_rcp = normp.tile([P, HB], F32, tag="trcp", name=f"trcp{h}")
                nc.vector.reciprocal(out=t_rcp, in_=t_ss)
                t_rsq = normp.tile([P, HB], F32, tag="trsq", name=f"trsq{h}")
                nc.scalar.activation(out=t_rsq, in_=t_rcp, func=AF.Sqrt)
                t_scl = normp.tile([P, HB], F32, tag="tscl2", name=f"tscl2_{h}")
                nc.vector.tensor_mul(
                    out=t_scl,
                    in0=t_rsq,
                    in1=mask_sb[:, SB + h * HB : SB + (h + 1) * HB],
                )
                # single broadcast multiply: t_sc[p, k, d] = t_nat[p, k, d] * t_scl[p, k]
                scl_b = t_scl.unsqueeze(2).broadcast_to([P, HB, D])
                nc.vector.tensor_mul(
                    out=t_sc[:, blk, :], in0=t_nat[:, blk, :], in1=scl_b
                )

            def t_xpose(g):
                ps = psum_t.tile([P, G * P], F32, tag="pst", name=f"ps_t{g}")
                for j in range(G):
                    k = g * G + j
                    nc.tensor.transpose(ps[:, j * P : (j + 1) * P], t_sc[:, k, :], ident)
                nc.vector.tensor_copy(
                    out=tT[:, g * G * P : (g + 1) * G * P], in_=ps
                )

            # --- main matmul + fused (scale * relu) -> fp16 + half-row DMA.
            # Two MMs fill a 2-bank PSUM tile; one 1024-wide copy drains it;
            # one 256KB DMA ships it. ACT takes `act_copy` of every 32 copies.
            ACT_COPY = frozenset(
                (0, 1, 3, 4, 6, 8, 9, 11, 12, 14, 16, 17, 19, 20, 22, 24, 25, 27, 28)
            )
            copy_idx = [0]

            def half_row(m, h):
                ps = psum_mm.tile([P, 2 * NT], F32, tag="psmm", name=f"mm{m}_{h}")
                for q in range(2):
                    n = 2 * h + q
                    nc.tensor.matmul(
                        ps[:, q * NT : (q + 1) * NT],
                        sT[:, m * P : (m + 1) * P],
                        tT[:, n * NT : (n + 1) * NT],
                        start=True,
                        stop=True,
                    )
                ob = outp.tile([P, 2 * NT], F16, tag="ob", name=f"ob{m}_{h}")
                i = copy_idx[0]
                copy_idx[0] += 1
                if i in ACT_COPY:
                    nc.scalar.activation(
                        out=ob, in_=ps, func=AF.Relu, scale=s_scl[:, m : m + 1]
                    )
                else:
                    nc.vector.tensor_scalar(
                        out=ob,
                        in0=ps,
                        scalar1=s_scl[:, m : m + 1],
                        scalar2=0.0,
                        op0=ALU.mult,
                        op1=ALU.max,
                    )
                nc.sync.dma_start(
                    out=out_r[m][:, h * 2 * NT : (h + 1) * 2 * NT], in_=ob
                )

            # Emission order == per-engine FIFO order. t half 0 is the
            # critical path to the first output half-rows; s transposes can
            # start as soon as each s group lands (no norm dependency).
            t_load(0)
            t_load(1)
            s_load(0)
            s_load(1)
            t_load(2)
            t_load(3)
            s_load(2)
            s_load(3)
            t_norm(0)
            s_xpose(0)
            t_xpose(0)
            s_norm(0)
            t_xpose(1)
            s_xpose(1)
            t_norm(1)
            # first half sweep: rows 0..15 over chunks 0,1
            half_row(0, 0)
            half_row(1, 0)
            s_xpose(2)
            half_row(2, 0)
            t_xpose(2)
            half_row(3, 0)
            s_xpose(3)
            half_row(4, 0)
            t_xpose(3)
            half_row(5, 0)
            s_norm(1)
            for m in range(6, SB):
                half_row(m, 0)
            for m in range(SB):
                half_row(m, 1)

    nc.compile()
    return nc


_NC_CACHE = None


def _get_nc():
    global _NC_CACHE
    if _NC_CACHE is None:
        _NC_CACHE = build_nc()
    return _NC_CACHE


def kernel(source, target, mask_src, mask_tar, **run_kwargs):
    source = np.asarray(source, dtype=np.float32)
    target = np.asarray(target, dtype=np.float32)
    mask_src = np.asarray(mask_src)
    mask_tar = np.asarray(mask_tar)
    B = source.shape[0]

    in_maps = []
    for b in range(B):
        # source tokens in (p k) order; target tokens in (k p) order.
        msf = mask_src[b].astype(np.float32).reshape(P, SB)
        mtf = mask_tar[b].astype(np.float32).reshape(TB, P).T
        mk = np.ascontiguousarray(np.concatenate([msf, mtf], axis=1))
        in_maps.append(
            {
                "src": np.ascontiguousarray(source[b]),
                "tgt": np.ascontiguousarray(target[b]),
                "maskf": mk,
            }
        )

    nc = _get_nc()
    res = run_bass_kernel_spmd(nc, in_maps, core_ids=list(range(B)), **run_kwargs)
    out = np.stack(
        [np.asarray(r["out"], dtype=np.float32) for r in res.results], axis=0
    )
    if run_kwargs.get("trace"):
        kernel.last_results = res
    return out
